# revision 10
# baseline (speedup 1.0000x reference)
"""LIF neuron scan kernel for Trainium2 (8 NeuronCores, raw Bass SPMD).

Math (per timestep): v = v_prev*0.5 + x + r; s = (v > 0); v *= (1-s).
Reset+leak fold to v = 0.5*min(v_prev, 0) + (x + r).  With block-local
power-of-two rescaling (block size K=10): within block, state w_i = 2^i*v
obeys the mult-free step  w_i = min(w_{i-1}, 0) + u_i  with
u_i = 2^i*(x+r) (prescaled on host; exact fp16 exponent shifts, values
bounded by ~2^10*20 << fp16 max).  At block boundaries the carried state
rescales by 2^-K:  w'_0 = min(w_9*2^-K + u_0, u_0)  (two fused DVE ops).
Inputs ship as fp16 (validated: rel err ~9e-3 vs the 2e-2 gate), halving
HBM traffic; spikes s = Sign(w) > 0 are unaffected by the scaling.

The serial time recurrence runs step-wise on the DVE as THREE interleaved
independent chains (feature thirds), so consecutive instructions have no
data dependency and the engine pipeline stays full (2-chain measured
latency-bound at 149ns/op; 3 chains target ~90ns/op).  GpSimd is kept
completely idle (its big ops starve concurrent DVE work; measured).

Sharding: data-parallel along batch; core i gets inp[:, 8i:8i+8, :].
Per-core layout [128 partitions, T, F=128 features] fp16, time-major.
Input DMAs alternate between both hardware DGE queues (SP and Act);
spikes (uint8 via Act's saturating Sign cast) DMA out on the Act queue.

Write-visibility discipline (observed on HW): an engine's posted SBUF
writes can lag its semaphore increment, so cross-engine consumers wait
with a one-producer-chunk lag (sign(c) waits dve_done >= c+2, final
chunk covered by DVE's trailing drain-inc; the out-DMA for chunk c is
issued after sign(c+1), tail after act.drain()).
"""
import sys
sys.path.insert(0, "/opt/trn_rl_repo")
import numpy as np
import concourse.bass as bass
from concourse import mybir
from concourse.bass_utils import run_bass_kernel_spmd

F16 = mybir.dt.float16
U8 = mybir.dt.uint8
T, B, N = 100, 64, 2048
NCORES = 8
B_LOC = B // NCORES
P = 128
F = (B_LOC * N) // P          # 128 features per partition
K = 10                        # rescale block size == DMA/sign chunk size
NCHUNK = T // K               # 10 DMA chunks
CF = K * F                    # 1280 elems per partition per chunk
HS = (0, 43, 86, 128)         # chain feature splits
SG = 5                        # sign/out piece: 5 timesteps
NSIGN = T // SG               # 20 pieces
SF = SG * F                   # 640 elems per piece
RESC = float(2.0 ** -K)
SP_CHUNKS = (0, 2, 4, 6, 8)
ACT_CHUNKS = (1, 3, 5, 7, 9)


def _build_nc():
    nc = bass.Bass()
    u_ext = nc.dram_tensor("u", [P, T * F], F16, kind="ExternalInput")
    s_ext = nc.dram_tensor("s", [P, T * F], U8, kind="ExternalOutput")

    with (
        nc.sbuf_tensor([P, T * F], F16) as ub,
        nc.sbuf_tensor([P, T * F], F16) as wb,
        nc.sbuf_tensor([P, T * F], U8) as sb,
        nc.sbuf_tensor([P, F], F16, side="right") as z0,
        nc.sbuf_tensor([P, F], F16, side="right") as tmp,
        nc.semaphore() as sem_sp,
        nc.semaphore() as sem_act,
        nc.semaphore() as dve_done,
        nc.semaphore() as sem_out,
        nc.Block(no_gpsimd_drain=True) as block,
    ):
        def in_dep(c):
            if c in SP_CHUNKS:
                return sem_sp, 16 * (SP_CHUNKS.index(c) + 1)
            return sem_act, 16 * (ACT_CHUNKS.index(c) + 1)

        @block.sync
        def _(sync):
            for c in SP_CHUNKS:
                lo = c * CF
                sync.dma_start(ub[:, lo:lo+CF], u_ext[:, lo:lo+CF]).then_inc(sem_sp, 16)

        @block.vector
        def _(vector):
            vector.memset(z0[:], 0.0)
            for c in range(NCHUNK):
                sem, cnt = in_dep(c)
                vector.wait_ge(sem, cnt)
                for i in range(K):
                    t = c * K + i
                    if i == 0 and c > 0:
                        # boundary: w = min(wprev*2^-K + u, u); emit all
                        # chains' stt halves first so the dependent mins
                        # are not back-to-back with their producers
                        for h in range(3):
                            sl = slice(t*F + HS[h], t*F + HS[h+1])
                            wprev = wb[:, (t-1)*F + HS[h]:(t-1)*F + HS[h+1]]
                            nc.vector.scalar_tensor_tensor(
                                tmp[:, HS[h]:HS[h+1]], wprev, RESC, ub[:, sl],
                                mybir.AluOpType.mult, mybir.AluOpType.add)
                        for h in range(3):
                            sl = slice(t*F + HS[h], t*F + HS[h+1])
                            nc.vector.tensor_tensor(
                                wb[:, sl], tmp[:, HS[h]:HS[h+1]], ub[:, sl],
                                mybir.AluOpType.min)
                        continue
                    for h in range(3):
                        sl = slice(t*F + HS[h], t*F + HS[h+1])
                        if t == 0:
                            wprev = z0[:, HS[h]:HS[h+1]]
                        else:
                            wprev = wb[:, (t-1)*F + HS[h]:(t-1)*F + HS[h+1]]
                        ins = nc.vector.scalar_tensor_tensor(
                            wb[:, sl], wprev, 0.0, ub[:, sl],
                            mybir.AluOpType.min, mybir.AluOpType.add)
                        if t % SG == SG - 1 and h == 2:
                            ins.then_inc(dve_done, 1)
            vector.maybe_drain_then_inc((dve_done, 1))

        @block.scalar
        def _(act):
            for c in ACT_CHUNKS:
                lo = c * CF
                act.dma_start(ub[:, lo:lo+CF], u_ext[:, lo:lo+CF]).then_inc(sem_act, 16)
            for i in range(NSIGN):
                lo = i * SF
                act.wait_ge(dve_done, min(i + 2, NSIGN + 1))
                nc.scalar.activation(sb[:, lo:lo+SF], wb[:, lo:lo+SF],
                                     mybir.ActivationFunctionType.Sign)
                if i >= 1:
                    o = i - 1
                    act.dma_start(s_ext[:, o*SF:(o+1)*SF], sb[:, o*SF:(o+1)*SF]).then_inc(sem_out, 16)
            act.drain()
            o = NSIGN - 1
            act.dma_start(s_ext[:, o*SF:(o+1)*SF], sb[:, o*SF:(o+1)*SF]).then_inc(sem_out, 16)

    return nc


# host prescale: u[t] = fp16(x+r) * 2^(t mod K)  (exact exponent shift)
_SCALE16 = np.exp2(np.arange(T, dtype=np.float32) % K).astype(np.float16)


def _shard(inp: np.ndarray, rec: np.ndarray) -> list[dict[str, np.ndarray]]:
    u16 = (inp + rec).astype(np.float16) * _SCALE16[:, None, None]
    in_maps = []
    for i in range(NCORES):
        uc = u16[:, i*B_LOC:(i+1)*B_LOC, :].reshape(T, P, F)
        in_maps.append({"u": np.ascontiguousarray(uc.transpose(1, 0, 2)).reshape(P, T * F)})
    return in_maps


def kernel(inp: np.ndarray, rec: np.ndarray) -> np.ndarray:
    inp = np.asarray(inp, dtype=np.float32)
    rec = np.asarray(rec, dtype=np.float32)
    nc = _build_nc()
    in_maps = _shard(inp, rec)
    res = run_bass_kernel_spmd(nc, in_maps, list(range(NCORES)))
    outs = []
    for i in range(NCORES):
        raw = res.results[i]["s"].reshape(P, T, F)           # uint8
        s = (raw == 1).astype(np.float32).transpose(1, 0, 2)  # [T, P, F]
        outs.append(s.reshape(T, B_LOC, N))
    return np.concatenate(outs, axis=1)


# revision 11
# speedup vs baseline: 1.1099x; 1.1099x over previous
"""LIF neuron scan kernel for Trainium2 (8 NeuronCores, raw Bass SPMD).

Math (per timestep, fp32): v = v_prev*0.5 + x + r; s = (v > 0); v *= (1-s).
Reset+leak fold to v = 0.5*min(v_prev, 0) + (x + r).  With the exact fp32
rescaling w_t = 2^t * v_t (power-of-two scaling commutes with IEEE rounding)
the recurrence becomes w_t = min(w_{t-1}, 0) + u_t with u_t = 2^t*(x_t+r_t)
(prescaled on host; max |w| ~ 2^103 << fp32 max), and s_t = (w_t > 0).

The whole 100-step recurrence runs as 8 hardware tensor_tensor_scan
instructions (DVE prefix scan along the free dim, measured ~2.16ns/elem):
    state = (zeros[:,i] min state) add u[:,i]
Layout per partition: R=128 feature rows of length 101 = [spacer, t0..t99],
time innermost.  The spacer value +2^126 forces state > 0, so the next
element sees min(state,0) = 0 -- the scan self-resets at every row
boundary, rows chunk independently (initial=0.0), and chunk sizes ramp
up ([4,8,16,20,...] rows) so the first scan starts as early as possible.

Spikes are Sign(w) on the scalar engine, emitted as uint8 (saturating
cast {-1,0,1}->{0,0,1}) in half-chunk pieces for a short tail; host
decodes raw==1 and drops the spacer column.  GpSimd stays completely
idle (its ops starve concurrent DVE work; measured 7x scan slowdown).

Sharding: data-parallel along batch; core i gets inp[:, 8i:8i+8, :].
Input DMAs split across both hardware DGE queues (SP and Act).

Write-visibility discipline (observed on HW): an engine's posted SBUF
writes can lag its semaphore increment, so cross-engine consumers wait
with a one-producer-chunk lag (sign pieces of chunk c wait dve_done >=
c+2, final chunk covered by the DVE trailing drain-inc; the out-DMA for
piece p is issued after sign piece p+1, tail after act.drain()).
"""
import sys
sys.path.insert(0, "/opt/trn_rl_repo")
import numpy as np
import concourse.bass as bass
from concourse import mybir
from concourse.bass_utils import run_bass_kernel_spmd

F32 = mybir.dt.float32
U8 = mybir.dt.uint8
T, B, N = 100, 64, 2048
NCORES = 8
B_LOC = B // NCORES
P = 128
R = (B_LOC * N) // P          # 128 feature rows per partition
L = T + 1                     # row length: [spacer, t0..t99]
FREE = R * L                  # 12928 elems per partition
SPACER = float(2.0 ** 126)
CHUNK_ROWS = (4, 8, 16, 20, 20, 20, 20, 20)   # ramp-up, sums to 128
NCHUNK = len(CHUNK_ROWS)
ROW_OFF = [sum(CHUNK_ROWS[:i]) for i in range(NCHUNK + 1)]
ZROWS = max(CHUNK_ROWS)
SP_CHUNKS = (0, 2, 4, 6)
ACT_CHUNKS = (1, 3, 5, 7)
# sign/out pieces: half chunks
PIECES = []                   # (row_lo, nrows, chunk)
for _c, _n in enumerate(CHUNK_ROWS):
    PIECES.append((ROW_OFF[_c], _n // 2, _c))
    PIECES.append((ROW_OFF[_c] + _n // 2, _n - _n // 2, _c))


def _build_nc():
    nc = bass.Bass()
    u_ext = nc.dram_tensor("u", [P, FREE], F32, kind="ExternalInput")
    s_ext = nc.dram_tensor("s", [P, FREE], U8, kind="ExternalOutput")

    with (
        nc.sbuf_tensor([P, FREE], F32) as ub,
        nc.sbuf_tensor([P, FREE], F32) as wb,
        nc.sbuf_tensor([P, FREE], U8) as sb,
        nc.sbuf_tensor([P, ZROWS * L], F32, side="right") as zb,
        nc.semaphore() as sem_sp,
        nc.semaphore() as sem_act,
        nc.semaphore() as dve_done,
        nc.semaphore() as sem_out,
        nc.Block(no_gpsimd_drain=True) as block,
    ):
        def in_dep(c):
            if c in SP_CHUNKS:
                return sem_sp, 16 * (SP_CHUNKS.index(c) + 1)
            return sem_act, 16 * (ACT_CHUNKS.index(c) + 1)

        @block.sync
        def _(sync):
            for c in SP_CHUNKS:
                lo, sz = ROW_OFF[c] * L, CHUNK_ROWS[c] * L
                sync.dma_start(ub[:, lo:lo+sz], u_ext[:, lo:lo+sz]).then_inc(sem_sp, 16)

        @block.vector
        def _(vector):
            vector.memset(zb[:], 0.0)
            for c in range(NCHUNK):
                lo, sz = ROW_OFF[c] * L, CHUNK_ROWS[c] * L
                sem, cnt = in_dep(c)
                vector.wait_ge(sem, cnt)
                nc.vector.tensor_tensor_scan(
                    wb[:, lo:lo+sz], zb[:, :sz], ub[:, lo:lo+sz],
                    0.0, mybir.AluOpType.min, mybir.AluOpType.add,
                ).then_inc(dve_done, 1)
            vector.maybe_drain_then_inc((dve_done, 1))

        @block.scalar
        def _(act):
            for c in ACT_CHUNKS:
                lo, sz = ROW_OFF[c] * L, CHUNK_ROWS[c] * L
                act.dma_start(ub[:, lo:lo+sz], u_ext[:, lo:lo+sz]).then_inc(sem_act, 16)
            for p, (rlo, nr, c) in enumerate(PIECES):
                lo, sz = rlo * L, nr * L
                act.wait_ge(dve_done, min(c + 2, NCHUNK + 1))
                nc.scalar.activation(sb[:, lo:lo+sz], wb[:, lo:lo+sz],
                                     mybir.ActivationFunctionType.Sign)
                if p >= 1:
                    olo, onr, _ = PIECES[p - 1]
                    act.dma_start(s_ext[:, olo*L:(olo+onr)*L],
                                  sb[:, olo*L:(olo+onr)*L]).then_inc(sem_out, 16)
            act.drain()
            olo, onr, _ = PIECES[-1]
            act.dma_start(s_ext[:, olo*L:(olo+onr)*L],
                          sb[:, olo*L:(olo+onr)*L]).then_inc(sem_out, 16)

    return nc


_SCALE = np.exp2(np.arange(T, dtype=np.float32)).astype(np.float32)


def _shard(inp: np.ndarray, rec: np.ndarray) -> list[dict[str, np.ndarray]]:
    u_all = (inp + rec) * _SCALE[:, None, None]
    in_maps = []
    for i in range(NCORES):
        uc = u_all[:, i*B_LOC:(i+1)*B_LOC, :].reshape(T, P * R)
        buf = np.empty((P, R, L), dtype=np.float32)
        buf[:, :, 0] = SPACER
        buf[:, :, 1:] = np.ascontiguousarray(uc.T).reshape(P, R, T)
        in_maps.append({"u": buf.reshape(P, FREE)})
    return in_maps


def kernel(inp: np.ndarray, rec: np.ndarray) -> np.ndarray:
    inp = np.asarray(inp, dtype=np.float32)
    rec = np.asarray(rec, dtype=np.float32)
    nc = _build_nc()
    in_maps = _shard(inp, rec)
    res = run_bass_kernel_spmd(nc, in_maps, list(range(NCORES)))
    outs = []
    for i in range(NCORES):
        raw = res.results[i]["s"].reshape(P, R, L)[:, :, 1:]   # drop spacers
        s = (raw == 1).astype(np.float32).reshape(P * R, T).T  # [T, P*R]
        outs.append(s.reshape(T, B_LOC, N))
    return np.concatenate(outs, axis=1)
